# revision 1
# baseline (speedup 1.0000x reference)
"""MHA kernel for TRN2: B=4,T=2048,D=1024,H=16,HD=64 across 8 NeuronCores.

Sharding: core c -> batch c//2, query half c%2 (host rotates the sequence so
each core's queries are rows 0:1024; softmax over keys is permutation
invariant). No collectives. All matmuls fp32r. Transposed-logits layout
(P^T [s,q]); a ones-column folded into V yields softmax denominators from the
same PV matmul; denominators are broadcast across partitions with a K=1 matmul.
"""
import sys
sys.path.insert(0, "/opt/trn_rl_repo")
import warnings
warnings.filterwarnings("ignore")

import numpy as np
import concourse.bass as bass
import concourse.mybir as mybir
import concourse.tile as tile
from concourse import bacc
from concourse.bass_utils import run_bass_kernel_spmd
from concourse.masks import make_identity

F32 = mybir.dt.float32
F32R = mybir.dt.float32r
EXP = mybir.ActivationFunctionType.Exp

T, D = 2048, 1024
TQ = 1024          # queries per core
NG = 8             # head groups (2 heads each)
NSC = 16           # s chunks of 128
NDC = 8            # d chunks of 128
SCALE = 0.125      # 1/sqrt(64)


def _transpose_8(nc, psw, nat, ident, copy_out):
    """Transpose nat [128,1024] in two 4-chunk batches; copy_out(half, psum_view)."""
    for half in range(2):
        p = psw.tile([128, 512], F32, tag="work")
        for k in range(4):
            dc = half * 4 + k
            nc.tensor.transpose(
                p[:, k * 128:(k + 1) * 128], nat[:, dc * 128:(dc + 1) * 128], ident)
        copy_out(half, p.rearrange("p (k f) -> p k f", k=4))


def build_nc():
    nc = bacc.Bacc("TRN2", target_bir_lowering=False, debug=False, num_devices=8)
    xin = nc.dram_tensor("xin", [T, D], F32, kind="ExternalInput")
    wq = nc.dram_tensor("wq", [D, D], F32, kind="ExternalInput")
    wk = nc.dram_tensor("wk", [D, D], F32, kind="ExternalInput")
    wv = nc.dram_tensor("wv", [D, D], F32, kind="ExternalInput")
    wo = nc.dram_tensor("wo", [D, D], F32, kind="ExternalInput")
    bo = nc.dram_tensor("bo", [1, D], F32, kind="ExternalInput")
    y = nc.dram_tensor("y", [TQ, D], F32, kind="ExternalOutput")

    with tile.TileContext(nc) as tc:
        with (
            tc.tile_pool(name="persist", bufs=1) as pp,
            tc.tile_pool(name="xtp", bufs=1) as xp,
            tc.tile_pool(name="vq", bufs=1) as vp,
            tc.tile_pool(name="wv1", bufs=1) as wvp,
            tc.tile_pool(name="work", bufs=2) as wp,
            tc.tile_pool(name="small", bufs=2) as sp,
            tc.tile_pool(name="ptp", bufs=3) as ptp,
            tc.tile_pool(name="ps_work", bufs=2, space="PSUM") as psw,
            tc.tile_pool(name="ps_pv", bufs=2, space="PSUM") as psv,
            tc.tile_pool(name="ps_log", bufs=2, space="PSUM") as psl,
        ):
            ident = pp.tile([128, 128], F32)
            make_identity(nc, ident)
            bias = pp.tile([128, D], F32)
            nc.sync.dma_start(
                out=bias, in_=bass.AP(tensor=bo, offset=0, ap=[[0, 128], [1, D]]))
            onesf = pp.tile([128, 64], F32)
            nc.vector.memset(onesf, 1.0)
            ones = pp.tile([65, 64], F32R)
            nc.vector.tensor_copy(out=ones, in_=onesf[0:65, :])
            catT = [pp.tile([128, TQ], F32R, tag=f"catT{g}", name=f"catT{g}")
                    for g in range(NG)]

            # ---- x^T : [128, dc, t] fp32r ----
            xT = xp.tile([128, NDC, T], F32R, tag="big")
            for tcb in range(NSC):
                nat = wp.tile([128, D], F32, tag="nat")
                nc.sync.dma_start(out=nat, in_=xin[tcb * 128:(tcb + 1) * 128, :])
                _transpose_8(nc, psw, nat, ident, lambda half, pv: nc.vector.tensor_copy(
                    out=xT[:, half * 4:half * 4 + 4, tcb * 128:(tcb + 1) * 128], in_=pv))

            vtile = None

            def build_vquarter(qid):
                """V for heads 4qid..4qid+3 -> [128 s, sc, 4 h, 65] (col 64 = ones)."""
                vt = vp.tile([128, NSC, 4, 65], F32R, tag="vq")
                wvT = wvp.tile([128, NDC, 256], F32R, tag="wvT")
                for rb in range(2):
                    nat = wp.tile([128, D], F32, tag="nat")
                    nc.sync.dma_start(
                        out=nat,
                        in_=wv[qid * 256 + rb * 128: qid * 256 + (rb + 1) * 128, :])
                    _transpose_8(nc, psw, nat, ident,
                                 lambda half, pv, rb=rb: nc.vector.tensor_copy(
                                     out=wvT[:, half * 4:half * 4 + 4,
                                             rb * 128:(rb + 1) * 128], in_=pv))
                for sc in range(NSC):
                    p = psw.tile([128, 512], F32, tag="work")
                    for dc in range(NDC):
                        nc.tensor.matmul(
                            p[:, 0:256], xT[:, dc, sc * 128:(sc + 1) * 128],
                            wvT[:, dc, :], start=(dc == 0), stop=(dc == NDC - 1))
                    nc.vector.tensor_copy(
                        out=vt[:, sc, :, 0:64],
                        in_=p[:, 0:256].rearrange("p (h c) -> p h c", h=4))
                nc.vector.tensor_copy(
                    out=vt[:, :, :, 64:65],
                    in_=onesf.rearrange("p (a b c) -> p a b c", a=NSC, b=4))
                return vt

            for g in range(NG):
                if g % 2 == 0:
                    vtile = build_vquarter(g // 2)
                i0 = 2 * (g % 2)  # head index within the quarter

                wqT = wp.tile([128, NDC, 128], F32R, tag="wqT")
                wkT = wp.tile([128, NDC, 128], F32R, tag="wkT")
                for (src, dst) in ((wq, wqT), (wk, wkT)):
                    nat = wp.tile([128, D], F32, tag="nat")
                    nc.sync.dma_start(out=nat, in_=src[g * 128:(g + 1) * 128, :])
                    _transpose_8(nc, psw, nat, ident,
                                 lambda half, pv, dst=dst: nc.vector.tensor_copy(
                                     out=dst[:, half * 4:half * 4 + 4, :], in_=pv))

                qT = wp.tile([128, TQ], F32R, tag="qT")
                for qh in range(2):
                    p = psw.tile([128, 512], F32, tag="work")
                    for dc in range(NDC):
                        nc.tensor.matmul(
                            p, wqT[:, dc, :], xT[:, dc, qh * 512:(qh + 1) * 512],
                            start=(dc == 0), stop=(dc == NDC - 1))
                    nc.vector.tensor_copy(out=qT[:, qh * 512:(qh + 1) * 512], in_=p)

                kT = wp.tile([128, T], F32R, tag="kT")
                for sb in range(4):
                    p = psw.tile([128, 512], F32, tag="work")
                    for dc in range(NDC):
                        nc.tensor.matmul(
                            p, wkT[:, dc, :], xT[:, dc, sb * 512:(sb + 1) * 512],
                            start=(dc == 0), stop=(dc == NDC - 1))
                    nc.vector.tensor_copy(out=kT[:, sb * 512:(sb + 1) * 512], in_=p)

                for qh in range(2):
                    qs = slice(qh * 512, (qh + 1) * 512)
                    pv0 = psv.tile([65, 512], F32, tag="pv")
                    pv1 = psv.tile([65, 512], F32, tag="pv")
                    for sc in range(NSC):
                        lg = psl.tile([128, 2, 512], F32, tag="log")
                        nc.tensor.matmul(
                            lg[:, 0, :], kT[0:64, sc * 128:(sc + 1) * 128],
                            qT[0:64, qs], start=True, stop=True)
                        nc.tensor.matmul(
                            lg[:, 1, :], kT[64:128, sc * 128:(sc + 1) * 128],
                            qT[64:128, qs], start=True, stop=True)
                        pt = ptp.tile([128, 2, 512], F32R, tag="pt")
                        nc.scalar.activation(
                            out=pt.rearrange("p a b -> p (a b)"),
                            in_=lg.rearrange("p a b -> p (a b)"),
                            func=EXP, scale=SCALE)
                        nc.tensor.matmul(
                            pv0, vtile[:, sc, i0, :], pt[:, 0, :],
                            start=(sc == 0), stop=(sc == NSC - 1))
                        nc.tensor.matmul(
                            pv1, vtile[:, sc, i0 + 1, :], pt[:, 1, :],
                            start=(sc == 0), stop=(sc == NSC - 1))
                    for hloc, pv in ((0, pv0), (1, pv1)):
                        # sums (row 64) -> broadcast to 64 partitions via K=1 matmul
                        s1 = sp.tile([65, 512], F32R, tag="s1")
                        nc.vector.tensor_copy(out=s1[64:65, :], in_=pv[64:65, :])
                        pb = psw.tile([128, 512], F32, tag="work")
                        nc.tensor.matmul(
                            pb[0:64, :], ones[64:65, :], s1[64:65, :],
                            start=True, stop=True)
                        rec = sp.tile([64, 512], F32, tag="rec")
                        nc.vector.reciprocal(out=rec, in_=pb[0:64, :])
                        if hloc == 0:
                            nc.vector.tensor_mul(
                                out=catT[g][0:64, qs], in0=pv[0:64, :], in1=rec)
                        else:
                            tmp = sp.tile([64, 512], F32R, tag="tmp")
                            nc.vector.tensor_mul(out=tmp, in0=pv[0:64, :], in1=rec)
                            nc.sync.dma_start(out=catT[g][64:128, qs], in_=tmp)

            # ---- final projection (woT reuses xT's slot) ----
            woT = xp.tile([128, NDC, D], F32R, tag="big")
            for rb in range(NDC):
                nat = wp.tile([128, D], F32, tag="nat")
                nc.sync.dma_start(out=nat, in_=wo[rb * 128:(rb + 1) * 128, :])
                _transpose_8(nc, psw, nat, ident,
                             lambda half, pv, rb=rb: nc.vector.tensor_copy(
                                 out=woT[:, half * 4:half * 4 + 4,
                                         rb * 128:(rb + 1) * 128], in_=pv))
            for qb in range(8):
                yt = wp.tile([128, D], F32, tag="yt")
                for nh in range(2):
                    p = psw.tile([128, 512], F32, tag="work")
                    for g in range(NG):
                        nc.tensor.matmul(
                            p, catT[g][:, qb * 128:(qb + 1) * 128],
                            woT[:, g, nh * 512:(nh + 1) * 512],
                            start=(g == 0), stop=(g == NG - 1))
                    nc.vector.tensor_add(
                        out=yt[:, nh * 512:(nh + 1) * 512], in0=p,
                        in1=bias[:, nh * 512:(nh + 1) * 512])
                nc.sync.dma_start(out=y[qb * 128:(qb + 1) * 128, :], in_=yt)

    nc.compile()
    return nc


_CACHE = {}


def kernel(x, Wq, Wk, Wv, Wo, bo):
    if "nc" not in _CACHE:
        _CACHE["nc"] = build_nc()
    nc = _CACHE["nc"]
    x = np.ascontiguousarray(x, dtype=np.float32)
    wq2 = np.ascontiguousarray(Wq.reshape(D, D), dtype=np.float32)
    wk2 = np.ascontiguousarray(Wk.reshape(D, D), dtype=np.float32)
    wv2 = np.ascontiguousarray(Wv.reshape(D, D), dtype=np.float32)
    wo2 = np.ascontiguousarray(Wo, dtype=np.float32)
    bo2 = np.ascontiguousarray(bo.reshape(1, D), dtype=np.float32)
    in_maps = []
    for c in range(8):
        b, h = c // 2, c % 2
        xin = x[b] if h == 0 else np.concatenate([x[b, TQ:], x[b, :TQ]], axis=0)
        in_maps.append({"xin": np.ascontiguousarray(xin), "wq": wq2, "wk": wk2,
                        "wv": wv2, "wo": wo2, "bo": bo2})
    res = run_bass_kernel_spmd(nc, in_maps, core_ids=list(range(8)))
    out = np.empty((4, T, D), dtype=np.float32)
    for c in range(8):
        b, h = c // 2, c % 2
        out[b, h * TQ:(h + 1) * TQ] = res.results[c]["y"]
    return out



# revision 7
# speedup vs baseline: 1.2348x; 1.2348x over previous
"""MHA kernel for TRN2: B=4,T=2048,D=1024,H=16,HD=64 across 8 NeuronCores.

Sharding: core c -> batch c//2, query half c%2 (host rotates the sequence so
each core's queries are columns 0:1024 of x^T; softmax over keys is
permutation invariant). No collectives.

v2: all transposes moved to the host (x^T and per-group-packed W^T are fed
directly), bf16 for K/Q/V/pt operands (fp32 PSUM accumulation), and the
per-head-group projection matmuls are interleaved with the ACT-bound softmax
so PE and ACT overlap. Logits for the two heads of a group run row-packed
(K=64 at partitions 0:64 / 64:128 -> concurrent PE tiles). A ones-column
folded into V yields softmax denominators from the same PV matmul;
denominators broadcast across partitions with a K=1 matmul.
"""
import sys
sys.path.insert(0, "/opt/trn_rl_repo")
import warnings
warnings.filterwarnings("ignore")

import numpy as np
import concourse.bass as bass
import concourse.mybir as mybir
import concourse.tile as tile
from concourse import bacc
from concourse.bass_utils import run_bass_kernel_spmd

F32 = mybir.dt.float32
F32R = mybir.dt.float32r
BF16 = mybir.dt.bfloat16
EXP = mybir.ActivationFunctionType.Exp

T, D = 2048, 1024
TQ = 1024          # queries per core
NG = 8             # head groups (2 heads each)
NSC = 16           # s chunks of 128
NDC = 8            # d chunks of 128
SCALE = 0.125      # 1/sqrt(64)


def build_nc():
    nc = bacc.Bacc("TRN2", target_bir_lowering=False, debug=False, num_devices=8)
    xt = nc.dram_tensor("xt", [D, T], F32R, kind="ExternalInput")       # x[b]^T rot
    wq = nc.dram_tensor("wq", [D, D], F32R, kind="ExternalInput")       # [g*128+p, (dc,hk)]
    wk = nc.dram_tensor("wk", [D, D], F32R, kind="ExternalInput")
    wv = nc.dram_tensor("wv", [256, 4096], F32R, kind="ExternalInput")  # [hh*128+p, (dc,c512)]
    wo = nc.dram_tensor("wo", [D, D], F32R, kind="ExternalInput")       # Wo^T
    bo = nc.dram_tensor("bo", [1, D], F32, kind="ExternalInput")
    y = nc.dram_tensor("y", [TQ, D], F32, kind="ExternalOutput")

    with tile.TileContext(nc) as tc:
        with (
            tc.tile_pool(name="persist", bufs=1) as pp,
            tc.tile_pool(name="xtp", bufs=1) as xp,
            tc.tile_pool(name="wqk", bufs=2) as wqkp,
            tc.tile_pool(name="wvp", bufs=1) as wvp,
            tc.tile_pool(name="qkt", bufs=2) as qktp,
            tc.tile_pool(name="vtp", bufs=2) as vtp,
            tc.tile_pool(name="small", bufs=1) as sp,
            tc.tile_pool(name="ptp", bufs=3) as ptp,
            tc.tile_pool(name="yp", bufs=2) as yp,
            tc.tile_pool(name="ps_work", bufs=2, space="PSUM") as psw,
            tc.tile_pool(name="ps_pv", bufs=2, space="PSUM") as psv,
            tc.tile_pool(name="ps_log", bufs=2, space="PSUM") as psl,
        ):
            bias = pp.tile([128, D], F32)
            nc.sync.dma_start(
                out=bias, in_=bass.AP(tensor=bo, offset=0, ap=[[0, 128], [1, D]]))
            onesf = pp.tile([65, 64], F32)
            nc.vector.memset(onesf, 1.0)
            ones = pp.tile([65, 64], F32R)
            nc.vector.tensor_copy(out=ones, in_=onesf)
            catT = pp.tile([128, NG, TQ], F32R, name="catT")

            # ---- x^T straight from HBM: [128, dc, t] ----
            xT = xp.tile([128, NDC, T], F32R, tag="xT")
            for dc in range(NDC):
                nc.sync.dma_start(out=xT[:, dc, :], in_=xt[dc * 128:(dc + 1) * 128, :])

            def load_wqk(g):
                wqT = wqkp.tile([128, NDC, 128], F32R, tag="wqT")
                wkT = wqkp.tile([128, NDC, 128], F32R, tag="wkT")
                nc.sync.dma_start(
                    out=wqT, in_=wq[g * 128:(g + 1) * 128, :])
                nc.sync.dma_start(
                    out=wkT, in_=wk[g * 128:(g + 1) * 128, :])
                return wqT, wkT

            def proj_qk(wqT, wkT):
                """QT [128,1024] and KT [128,2048] (bf16) for one head group."""
                qt = qktp.tile([128, TQ], BF16, tag="qt")
                kt = qktp.tile([128, T], BF16, tag="kt")
                for qc in range(2):
                    p = psw.tile([128, 512], F32, tag="work")
                    for dc in range(NDC):
                        nc.tensor.matmul(
                            p, wqT[:, dc, :], xT[:, dc, qc * 512:(qc + 1) * 512],
                            start=(dc == 0), stop=(dc == NDC - 1))
                    nc.vector.tensor_copy(out=qt[:, qc * 512:(qc + 1) * 512], in_=p)
                for sb in range(4):
                    p = psw.tile([128, 512], F32, tag="work")
                    for dc in range(NDC):
                        nc.tensor.matmul(
                            p, wkT[:, dc, :], xT[:, dc, sb * 512:(sb + 1) * 512],
                            start=(dc == 0), stop=(dc == NDC - 1))
                    nc.vector.tensor_copy(out=kt[:, sb * 512:(sb + 1) * 512], in_=p)
                return qt, kt

            def load_wv(hh):
                wvT = wvp.tile([128, NDC, 512], F32R, tag="wvT")
                nc.sync.dma_start(out=wvT, in_=wv[hh * 128:(hh + 1) * 128, :])
                return wvT

            def new_vhalf():
                """V for 8 heads -> [128 s, sc, 8 h, 65] (col 64 = ones)."""
                vt = vtp.tile([128, NSC, 8, 65], BF16, tag="vt")
                nc.vector.memset(vt[:, :, :, 64:65], 1.0)
                return vt

            def build_vhalf(vt, wvT, sc_lo, sc_hi):
                for sc in range(sc_lo, sc_hi):
                    p = psw.tile([128, 512], F32, tag="work")
                    for dc in range(NDC):
                        nc.tensor.matmul(
                            p, xT[:, dc, sc * 128:(sc + 1) * 128], wvT[:, dc, :],
                            start=(dc == 0), stop=(dc == NDC - 1))
                    nc.vector.tensor_copy(
                        out=vt[:, sc, :, 0:64],
                        in_=p.rearrange("p (h c) -> p h c", h=8))

            def attention(g, qt, kt, vt):
                for qh in range(2):
                    qs = slice(qh * 512, (qh + 1) * 512)
                    pv0 = psv.tile([65, 512], F32, tag="pv")
                    pv1 = psv.tile([65, 512], F32, tag="pv")
                    for sc in range(NSC):
                        lg = psl.tile([128, 2, 512], F32, tag="log")
                        nc.tensor.matmul(
                            lg[:, 0, :], kt[0:64, sc * 128:(sc + 1) * 128],
                            qt[0:64, qs], start=True, stop=True)
                        nc.tensor.matmul(
                            lg[:, 1, :], kt[64:128, sc * 128:(sc + 1) * 128],
                            qt[64:128, qs], start=True, stop=True)
                        pt = ptp.tile([128, 2, 512], BF16, tag="pt")
                        nc.scalar.activation(
                            out=pt.rearrange("p a b -> p (a b)"),
                            in_=lg.rearrange("p a b -> p (a b)"),
                            func=EXP, scale=SCALE)
                        j = 2 * (g % 4)
                        nc.tensor.matmul(
                            pv0, vt[:, sc, j, :], pt[:, 0, :],
                            start=(sc == 0), stop=(sc == NSC - 1))
                        nc.tensor.matmul(
                            pv1, vt[:, sc, j + 1, :], pt[:, 1, :],
                            start=(sc == 0), stop=(sc == NSC - 1))
                    for hloc, pv in ((0, pv0), (1, pv1)):
                        # sums (row 64) -> broadcast to 64 partitions via K=1 matmul
                        s1 = sp.tile([65, 512], F32R, tag="s1")
                        nc.vector.tensor_copy(out=s1[64:65, :], in_=pv[64:65, :])
                        pb = psw.tile([128, 512], F32, tag="work")
                        nc.tensor.matmul(
                            pb[0:64, :], ones[64:65, :], s1[64:65, :],
                            start=True, stop=True)
                        rec = sp.tile([64, 512], F32, tag="rec")
                        nc.vector.reciprocal(out=rec, in_=pb[0:64, :])
                        if hloc == 0:
                            nc.vector.tensor_mul(
                                out=catT[0:64, g, qs], in0=pv[0:64, :], in1=rec)
                        else:
                            tmp = sp.tile([64, 512], F32R, tag="tmp")
                            nc.vector.tensor_mul(out=tmp, in0=pv[0:64, :], in1=rec)
                            nc.sync.dma_start(out=catT[64:128, g, qs], in_=tmp)

            # ---- prologue: V half 0, QT/KT group 0 ----
            wvT = load_wv(0)
            wq0 = load_wqk(0)
            vt = new_vhalf()
            build_vhalf(vt, wvT, 0, NSC)
            qt, kt = proj_qk(*wq0)

            woT = None
            vt_next = None
            for g in range(NG):
                wq_next = load_wqk(g + 1) if g < NG - 1 else None
                if g == 1:
                    wvT = load_wv(1)
                    vt_next = new_vhalf()
                if g == NG - 1:
                    # wo reuses xT's slot (xT's last reader is g=7's qk proj,
                    # emitted at the end of g=6); overlaps tail attention
                    woT = xp.tile([128, NG, D], F32R, tag="xT")
                    for gg in range(NG):
                        nc.sync.dma_start(
                            out=woT[:, gg, :], in_=wo[gg * 128:(gg + 1) * 128, :])
                attention(g, qt, kt, vt)
                if g == 2:
                    build_vhalf(vt_next, wvT, 0, NSC // 2)
                if g == 3:
                    build_vhalf(vt_next, wvT, NSC // 2, NSC)
                    vt = vt_next
                if g < NG - 1:
                    qt, kt = proj_qk(*wq_next)

            # ---- final projection ----
            for qb in range(8):
                yt = yp.tile([128, D], F32, tag="yt")
                for nh in range(2):
                    p = psw.tile([128, 512], F32, tag="work")
                    for gg in range(NG):
                        nc.tensor.matmul(
                            p, catT[:, gg, qb * 128:(qb + 1) * 128],
                            woT[:, gg, nh * 512:(nh + 1) * 512],
                            start=(gg == 0), stop=(gg == NG - 1))
                    nc.vector.tensor_add(
                        out=yt[:, nh * 512:(nh + 1) * 512], in0=p,
                        in1=bias[:, nh * 512:(nh + 1) * 512])
                nc.sync.dma_start(out=y[qb * 128:(qb + 1) * 128, :], in_=yt)

    nc.compile()
    return nc


_CACHE = {}


def _make_in_maps(ins):
    x = np.ascontiguousarray(ins["x"], dtype=np.float32)
    Wq2 = np.asarray(ins["Wq"], dtype=np.float32).reshape(D, D)
    Wk2 = np.asarray(ins["Wk"], dtype=np.float32).reshape(D, D)
    Wv2 = np.asarray(ins["Wv"], dtype=np.float32).reshape(D, D)
    Wo2 = np.asarray(ins["Wo"], dtype=np.float32)
    # per-group packed W^T: wq_r[g*128+p, dc*128+j] = Wq2[g*128+j, dc*128+p]
    wq_r = np.ascontiguousarray(
        Wq2.reshape(8, 128, 8, 128).transpose(0, 3, 2, 1).reshape(D, D))
    wk_r = np.ascontiguousarray(
        Wk2.reshape(8, 128, 8, 128).transpose(0, 3, 2, 1).reshape(D, D))
    # wv_r[hh*128+p, dc*512+c] = Wv2[hh*512+c, dc*128+p]
    wv_r = np.ascontiguousarray(
        Wv2.reshape(2, 512, 8, 128).transpose(0, 3, 2, 1).reshape(256, 4096))
    wo_r = np.ascontiguousarray(Wo2.T)
    bo2 = np.ascontiguousarray(
        np.asarray(ins["bo"], dtype=np.float32).reshape(1, D))
    xT = np.ascontiguousarray(x.transpose(0, 2, 1))  # [4, 1024, 2048]
    in_maps = []
    for c in range(8):
        b, h = c // 2, c % 2
        if h == 0:
            xtc = xT[b]
        else:
            xtc = np.ascontiguousarray(
                np.concatenate([xT[b][:, TQ:], xT[b][:, :TQ]], axis=1))
        in_maps.append({"xt": xtc, "wq": wq_r, "wk": wk_r, "wv": wv_r,
                        "wo": wo_r, "bo": bo2})
    return in_maps


def kernel(x, Wq, Wk, Wv, Wo, bo):
    if "nc" not in _CACHE:
        _CACHE["nc"] = build_nc()
    nc = _CACHE["nc"]
    in_maps = _make_in_maps(
        {"x": x, "Wq": Wq, "Wk": Wk, "Wv": Wv, "Wo": Wo, "bo": bo})
    res = run_bass_kernel_spmd(nc, in_maps, core_ids=list(range(8)))
    out = np.empty((4, T, D), dtype=np.float32)
    for c in range(8):
        b, h = c // 2, c % 2
        out[b, h * TQ:(h + 1) * TQ] = res.results[c]["y"]
    return out


# revision 9
# speedup vs baseline: 1.3265x; 1.0743x over previous
"""MHA kernel for TRN2: B=4,T=2048,D=1024,H=16,HD=64 across 8 NeuronCores.

Sharding: core c -> batch c//2, query half c%2 (host rotates the sequence so
each core's queries are columns 0:1024 of x^T; softmax over keys is
permutation invariant). No collectives.

v3: all-bf16 data path (host converts x and pre-transposed/packed weights to
bf16; fp32 PSUM accumulation everywhere), projection matmuls for the next
head group emitted as filler units inside the softmax-bound attention loop so
PE fills ACT-wait gaps, stationary-reuse matmul pairs (one LDWEIGHTS per two
matmuls), and a short off-PE denominator chain (reciprocal on the sums row,
then a K=1 broadcast matmul). Logits for the two heads of a group run
row-packed (K=64 at partitions 0:64 / 64:128 -> concurrent PE tiles). A
ones-column folded into V yields softmax denominators from the same PV
matmul.
"""
import sys
sys.path.insert(0, "/opt/trn_rl_repo")
import warnings
warnings.filterwarnings("ignore")

import numpy as np
import ml_dtypes
import concourse.bass as bass
import concourse.mybir as mybir
import concourse.tile as tile
from concourse import bacc
from concourse.bass_utils import run_bass_kernel_spmd

F32 = mybir.dt.float32
BF16 = mybir.dt.bfloat16
EXP = mybir.ActivationFunctionType.Exp

T, D = 2048, 1024
TQ = 1024          # queries per core
NG = 8             # head groups (2 heads each)
NSC = 16           # s chunks of 128
NDC = 8            # d chunks of 128
SCALE = 0.125      # 1/sqrt(64)


def build_nc():
    nc = bacc.Bacc("TRN2", target_bir_lowering=False, debug=False, num_devices=8)
    xt = nc.dram_tensor("xt", [D, T], BF16, kind="ExternalInput")       # x[b]^T rot
    wq = nc.dram_tensor("wq", [D, D], BF16, kind="ExternalInput")       # [g*128+p, (dc,hk)]
    wk = nc.dram_tensor("wk", [D, D], BF16, kind="ExternalInput")
    wv = nc.dram_tensor("wv", [256, 4096], BF16, kind="ExternalInput")  # [hh*128+p, (dc,c512)]
    wo = nc.dram_tensor("wo", [D, D], BF16, kind="ExternalInput")       # Wo^T
    bo = nc.dram_tensor("bo", [1, D], F32, kind="ExternalInput")
    y = nc.dram_tensor("y", [TQ, D], F32, kind="ExternalOutput")

    with tile.TileContext(nc) as tc:
        with (
            tc.tile_pool(name="persist", bufs=1) as pp,
            tc.tile_pool(name="xtp", bufs=1) as xp,
            tc.tile_pool(name="wqk", bufs=2) as wqkp,
            tc.tile_pool(name="wvp", bufs=1) as wvp,
            tc.tile_pool(name="wop", bufs=1) as wop,
            tc.tile_pool(name="qkt", bufs=2) as qktp,
            tc.tile_pool(name="vtp", bufs=2) as vtp,
            tc.tile_pool(name="small", bufs=2) as sp,
            tc.tile_pool(name="ptp", bufs=3) as ptp,
            tc.tile_pool(name="yp", bufs=2) as yp,
            tc.tile_pool(name="ps_work", bufs=2, space="PSUM") as psw,
            tc.tile_pool(name="ps_pv", bufs=2, space="PSUM") as psv,
            tc.tile_pool(name="ps_log", bufs=2, space="PSUM") as psl,
        ):
            bias = pp.tile([128, D], F32)
            nc.sync.dma_start(
                out=bias, in_=bass.AP(tensor=bo, offset=0, ap=[[0, 128], [1, D]]))
            onesf = pp.tile([65, 64], F32)
            nc.vector.memset(onesf, 1.0)
            ones = pp.tile([65, 64], BF16)
            nc.vector.tensor_copy(out=ones, in_=onesf)
            catT = pp.tile([128, NG, TQ], BF16, name="catT")

            # ---- x^T straight from HBM: [128, dc, t]; t-half chunks so
            # downstream matmuls start before the whole load lands ----
            xT = xp.tile([128, NDC, T], BF16, tag="xT")
            for th in range(2):
                for dc in range(NDC):
                    nc.sync.dma_start(
                        out=xT[:, dc, th * 1024:(th + 1) * 1024],
                        in_=xt[dc * 128:(dc + 1) * 128, th * 1024:(th + 1) * 1024])

            woT = wop.tile([128, NG, D], BF16, tag="woT")

            def load_wqk(g):
                wqT = wqkp.tile([128, NDC, 128], BF16, tag="wqT")
                wkT = wqkp.tile([128, NDC, 128], BF16, tag="wkT")
                nc.sync.dma_start(out=wqT, in_=wq[g * 128:(g + 1) * 128, :])
                nc.sync.dma_start(out=wkT, in_=wk[g * 128:(g + 1) * 128, :])
                return wqT, wkT

            def proj_unit(wT, dst, w0, w1):
                """Two 512-wide output windows sharing each stationary load."""
                p0 = psw.tile([128, 512], F32, tag="work")
                p1 = psw.tile([128, 512], F32, tag="work")
                for dc in range(NDC):
                    nc.tensor.matmul(
                        p0, wT[:, dc, :], xT[:, dc, w0 * 512:(w0 + 1) * 512],
                        start=(dc == 0), stop=(dc == NDC - 1))
                    nc.tensor.matmul(
                        p1, wT[:, dc, :], xT[:, dc, w1 * 512:(w1 + 1) * 512],
                        start=(dc == 0), stop=(dc == NDC - 1))
                nc.vector.tensor_copy(out=dst[:, w0 * 512:(w0 + 1) * 512], in_=p0)
                nc.vector.tensor_copy(out=dst[:, w1 * 512:(w1 + 1) * 512], in_=p1)

            def load_wv(hh):
                wvT = wvp.tile([128, NDC, 512], BF16, tag="wvT")
                nc.sync.dma_start(out=wvT, in_=wv[hh * 128:(hh + 1) * 128, :])
                return wvT

            def new_vhalf():
                """V for 8 heads -> [128 s, sc, 8 h, 65] (col 64 = ones)."""
                vt = vtp.tile([128, NSC, 8, 65], BF16, tag="vt")
                nc.vector.memset(vt[:, :, :, 64:65], 1.0)
                return vt

            def v_unit(vt, wvT, sc):
                p = psw.tile([128, 512], F32, tag="work")
                for dc in range(NDC):
                    nc.tensor.matmul(
                        p, xT[:, dc, sc * 128:(sc + 1) * 128], wvT[:, dc, :],
                        start=(dc == 0), stop=(dc == NDC - 1))
                nc.vector.tensor_copy(
                    out=vt[:, sc, :, 0:64],
                    in_=p.rearrange("p (h c) -> p h c", h=8))

            def attention(g, qt, kt, vt, fillers):
                fillers = list(fillers)
                fi = 0
                for qh in range(2):
                    qs = slice(qh * 512, (qh + 1) * 512)
                    pv0 = psv.tile([65, 512], F32, tag="pv")
                    pv1 = psv.tile([65, 512], F32, tag="pv")
                    for sc in range(NSC):
                        lg = psl.tile([128, 2, 512], F32, tag="log")
                        nc.tensor.matmul(
                            lg[:, 0, :], kt[0:64, sc * 128:(sc + 1) * 128],
                            qt[0:64, qs], start=True, stop=True)
                        nc.tensor.matmul(
                            lg[:, 1, :], kt[64:128, sc * 128:(sc + 1) * 128],
                            qt[64:128, qs], start=True, stop=True)
                        pt = ptp.tile([128, 2, 512], BF16, tag="pt")
                        nc.scalar.activation(
                            out=pt.rearrange("p a b -> p (a b)"),
                            in_=lg.rearrange("p a b -> p (a b)"),
                            func=EXP, scale=SCALE)
                        j = 2 * (g % 4)
                        nc.tensor.matmul(
                            pv0, vt[:, sc, j, :], pt[:, 0, :],
                            start=(sc == 0), stop=(sc == NSC - 1))
                        nc.tensor.matmul(
                            pv1, vt[:, sc, j + 1, :], pt[:, 1, :],
                            start=(sc == 0), stop=(sc == NSC - 1))
                        if sc % 3 == 2 and fi < len(fillers):
                            fillers[fi]()
                            fi += 1
                    for hloc, pv in ((0, pv0), (1, pv1)):
                        # 1/sum on the sums row, then broadcast to 64
                        # partitions with a K=1 matmul (psl bank; psw stays
                        # free for the projection fillers)
                        rec1 = sp.tile([65, 512], BF16, tag="rec1")
                        with nc.allow_low_precision(
                                reason="softmax denom reciprocal fits bf16"):
                            nc.vector.reciprocal(
                                out=rec1[64:65, :], in_=pv[64:65, :])
                        pbt = psl.tile([128, 2, 512], F32, tag="log")
                        pb = pbt[:, 0, :]
                        nc.tensor.matmul(
                            pb[0:64, :], ones[64:65, :], rec1[64:65, :],
                            start=True, stop=True)
                        rec = sp.tile([64, 512], F32, tag="rec")
                        nc.vector.tensor_copy(out=rec, in_=pb[0:64, :])
                        if hloc == 0:
                            nc.vector.tensor_mul(
                                out=catT[0:64, g, qs], in0=pv[0:64, :], in1=rec)
                        else:
                            tmp = sp.tile([64, 512], BF16, tag="tmp")
                            nc.vector.tensor_mul(out=tmp, in0=pv[0:64, :], in1=rec)
                            nc.sync.dma_start(out=catT[64:128, g, qs], in_=tmp)
                while fi < len(fillers):
                    fillers[fi]()
                    fi += 1

            # ---- prologue: V half 0, QT/KT group 0 ----
            wvT = load_wv(0)
            wq0, wk0 = load_wqk(0)
            vt = new_vhalf()
            for sc in range(NSC):
                v_unit(vt, wvT, sc)
            qt = qktp.tile([128, TQ], BF16, tag="qt")
            kt = qktp.tile([128, T], BF16, tag="kt")
            proj_unit(wq0, qt, 0, 1)
            proj_unit(wk0, kt, 0, 1)
            proj_unit(wk0, kt, 2, 3)
            for gg in range(NG):
                nc.sync.dma_start(
                    out=woT[:, gg, :], in_=wo[gg * 128:(gg + 1) * 128, :])

            vt_next = None
            for g in range(NG):
                fillers = []
                if g < NG - 1:
                    wqn, wkn = load_wqk(g + 1)
                    qt_n = qktp.tile([128, TQ], BF16, tag="qt")
                    kt_n = qktp.tile([128, T], BF16, tag="kt")
                    fillers.append(lambda w=wqn, d=qt_n: proj_unit(w, d, 0, 1))
                    fillers.append(lambda w=wkn, d=kt_n: proj_unit(w, d, 0, 1))
                    fillers.append(lambda w=wkn, d=kt_n: proj_unit(w, d, 2, 3))
                if g == 1:
                    wvT = load_wv(1)
                    vt_next = new_vhalf()
                if g in (2, 3):
                    lo = 0 if g == 2 else NSC // 2
                    for sc in range(lo, lo + NSC // 2):
                        fillers.append(lambda v=vt_next, w=wvT, s=sc: v_unit(v, w, s))
                attention(g, qt, kt, vt, fillers)
                if g == 3:
                    vt = vt_next
                if g < NG - 1:
                    qt, kt = qt_n, kt_n

            # ---- final projection (stationary catT chunk shared by 2 MMs) ----
            for qb in range(8):
                yt = yp.tile([128, D], F32, tag="yt")
                p0 = psw.tile([128, 512], F32, tag="work")
                p1 = psw.tile([128, 512], F32, tag="work")
                for gg in range(NG):
                    nc.tensor.matmul(
                        p0, catT[:, gg, qb * 128:(qb + 1) * 128],
                        woT[:, gg, 0:512], start=(gg == 0), stop=(gg == NG - 1))
                    nc.tensor.matmul(
                        p1, catT[:, gg, qb * 128:(qb + 1) * 128],
                        woT[:, gg, 512:1024], start=(gg == 0), stop=(gg == NG - 1))
                nc.vector.tensor_add(out=yt[:, 0:512], in0=p0, in1=bias[:, 0:512])
                nc.vector.tensor_add(out=yt[:, 512:1024], in0=p1, in1=bias[:, 512:1024])
                nc.sync.dma_start(out=y[qb * 128:(qb + 1) * 128, :], in_=yt)

    nc.compile()
    return nc


_CACHE = {}


def _make_in_maps(ins):
    bf = ml_dtypes.bfloat16
    x = np.asarray(ins["x"], dtype=np.float32)
    Wq2 = np.asarray(ins["Wq"], dtype=np.float32).reshape(D, D)
    Wk2 = np.asarray(ins["Wk"], dtype=np.float32).reshape(D, D)
    Wv2 = np.asarray(ins["Wv"], dtype=np.float32).reshape(D, D)
    Wo2 = np.asarray(ins["Wo"], dtype=np.float32)
    # per-group packed W^T: wq_r[g*128+p, dc*128+j] = Wq2[g*128+j, dc*128+p]
    wq_r = np.ascontiguousarray(
        Wq2.reshape(8, 128, 8, 128).transpose(0, 3, 2, 1).reshape(D, D).astype(bf))
    wk_r = np.ascontiguousarray(
        Wk2.reshape(8, 128, 8, 128).transpose(0, 3, 2, 1).reshape(D, D).astype(bf))
    # wv_r[hh*128+p, dc*512+c] = Wv2[hh*512+c, dc*128+p]
    wv_r = np.ascontiguousarray(
        Wv2.reshape(2, 512, 8, 128).transpose(0, 3, 2, 1).reshape(256, 4096).astype(bf))
    wo_r = np.ascontiguousarray(Wo2.T.astype(bf))
    bo2 = np.ascontiguousarray(
        np.asarray(ins["bo"], dtype=np.float32).reshape(1, D))
    xT = x.transpose(0, 2, 1).astype(bf)  # [4, 1024, 2048]
    in_maps = []
    for c in range(8):
        b, h = c // 2, c % 2
        if h == 0:
            xtc = np.ascontiguousarray(xT[b])
        else:
            xtc = np.ascontiguousarray(
                np.concatenate([xT[b][:, TQ:], xT[b][:, :TQ]], axis=1))
        in_maps.append({"xt": xtc, "wq": wq_r, "wk": wk_r, "wv": wv_r,
                        "wo": wo_r, "bo": bo2})
    return in_maps


def kernel(x, Wq, Wk, Wv, Wo, bo):
    if "nc" not in _CACHE:
        _CACHE["nc"] = build_nc()
    nc = _CACHE["nc"]
    in_maps = _make_in_maps(
        {"x": x, "Wq": Wq, "Wk": Wk, "Wv": Wv, "Wo": Wo, "bo": bo})
    res = run_bass_kernel_spmd(nc, in_maps, core_ids=list(range(8)))
    out = np.empty((4, T, D), dtype=np.float32)
    for c in range(8):
        b, h = c // 2, c % 2
        out[b, h * TQ:(h + 1) * TQ] = res.results[c]["y"]
    return out


# revision 12
# speedup vs baseline: 1.6747x; 1.2625x over previous
"""MHA kernel for TRN2: B=4,T=2048,D=1024,H=16,HD=64 across 8 NeuronCores.

Sharding: core c -> batch c//2, query half c%2 (host rotates the sequence so
each core's queries are columns 0:1024 of x^T; softmax over keys is
permutation invariant). No collectives.

v4: all-bf16 data path (host converts x and pre-transposed/packed weights to
bf16; fp32 PSUM accumulation everywhere); projection matmuls for the next
head group emitted as single-PSUM-slot filler units inside the softmax-bound
attention loop so PE fills ACT-wait gaps; logits for the two heads of a group
run row-packed (K=64 at partitions 0:64 / 64:128 -> concurrent PE tiles).
A ones-column folded FIRST into V yields softmax denominators in row 0 of the
same PV matmul; the normalize chain is PE- and PSUM-free: DVE reciprocal on
row 0, gpsimd partition_broadcast, DVE multiply, DMA partition-shift into the
concat layout.
"""
import sys
sys.path.insert(0, "/opt/trn_rl_repo")
import warnings
warnings.filterwarnings("ignore")

import numpy as np
import ml_dtypes
import concourse.bass as bass
import concourse.mybir as mybir
import concourse.tile as tile
from concourse import bacc
from concourse.bass_utils import run_bass_kernel_spmd

F32 = mybir.dt.float32
BF16 = mybir.dt.bfloat16
EXP = mybir.ActivationFunctionType.Exp

T, D = 2048, 1024
TQ = 1024          # queries per core
NG = 8             # head groups (2 heads each)
NSC = 16           # s chunks of 128
NDC = 8            # d chunks of 128
SCALE = 0.125      # 1/sqrt(64)


def build_nc():
    nc = bacc.Bacc("TRN2", target_bir_lowering=False, debug=False, num_devices=8)
    xt = nc.dram_tensor("xt", [D, T], BF16, kind="ExternalInput")       # x[b]^T rot
    wq = nc.dram_tensor("wq", [D, D], BF16, kind="ExternalInput")       # [g*128+p, (dc,hk)]
    wk = nc.dram_tensor("wk", [D, D], BF16, kind="ExternalInput")
    wv = nc.dram_tensor("wv", [256, 4096], BF16, kind="ExternalInput")  # [hh*128+p, (dc,c512)]
    wo = nc.dram_tensor("wo", [D, D], BF16, kind="ExternalInput")       # Wo^T
    bo = nc.dram_tensor("bo", [1, D], F32, kind="ExternalInput")
    y = nc.dram_tensor("y", [TQ, D], F32, kind="ExternalOutput")

    with tile.TileContext(nc) as tc:
        with (
            tc.tile_pool(name="persist", bufs=1) as pp,
            tc.tile_pool(name="xtp", bufs=1) as xp,
            tc.tile_pool(name="wqk", bufs=2) as wqkp,
            tc.tile_pool(name="wvp", bufs=1) as wvp,
            tc.tile_pool(name="wop", bufs=1) as wop,
            tc.tile_pool(name="qkt", bufs=2) as qktp,
            tc.tile_pool(name="vtp", bufs=2) as vtp,
            tc.tile_pool(name="small", bufs=2) as sp,
            tc.tile_pool(name="ptp", bufs=3) as ptp,
            tc.tile_pool(name="yp", bufs=2) as yp,
            tc.tile_pool(name="ps_work", bufs=2, space="PSUM") as psw,
            tc.tile_pool(name="ps_pv", bufs=2, space="PSUM") as psv,
            tc.tile_pool(name="ps_log", bufs=2, space="PSUM") as psl,
        ):
            # ---- weight/bias loads first: small, unblock the first groups ----
            wvT = wvp.tile([128, NDC, 512], BF16, tag="wvT")
            nc.sync.dma_start(out=wvT, in_=wv[0:128, :])
            wq0 = wqkp.tile([128, NDC, 128], BF16, tag="wqT")
            wk0 = wqkp.tile([128, NDC, 128], BF16, tag="wkT")
            nc.sync.dma_start(out=wq0, in_=wq[0:128, :])
            nc.sync.dma_start(out=wk0, in_=wk[0:128, :])

            # ---- x^T: [128, dc, t]; t-half chunks so downstream matmuls
            # start before the whole load lands ----
            xT = xp.tile([128, NDC, T], BF16, tag="xT")
            for th in range(2):
                for dc in range(NDC):
                    nc.sync.dma_start(
                        out=xT[:, dc, th * 1024:(th + 1) * 1024],
                        in_=xt[dc * 128:(dc + 1) * 128, th * 1024:(th + 1) * 1024])

            bias = pp.tile([128, D], F32)
            nc.sync.dma_start(
                out=bias, in_=bass.AP(tensor=bo, offset=0, ap=[[0, 128], [1, D]]))
            catT = pp.tile([128, NG, TQ], BF16, name="catT")
            woT = wop.tile([128, NG, D], BF16, tag="woT")
            for gg in range(NG):
                nc.sync.dma_start(
                    out=woT[:, gg, :], in_=wo[gg * 128:(gg + 1) * 128, :])

            def load_wqk(g):
                wqT = wqkp.tile([128, NDC, 128], BF16, tag="wqT")
                wkT = wqkp.tile([128, NDC, 128], BF16, tag="wkT")
                nc.sync.dma_start(out=wqT, in_=wq[g * 128:(g + 1) * 128, :])
                nc.sync.dma_start(out=wkT, in_=wk[g * 128:(g + 1) * 128, :])
                return wqT, wkT

            def proj_unit(wT, dst, w):
                """One 512-wide output window; holds a single psw slot."""
                p = psw.tile([128, 512], F32, tag="work")
                for dc in range(NDC):
                    nc.tensor.matmul(
                        p, wT[:, dc, :], xT[:, dc, w * 512:(w + 1) * 512],
                        start=(dc == 0), stop=(dc == NDC - 1))
                nc.vector.tensor_copy(out=dst[:, w * 512:(w + 1) * 512], in_=p)

            def new_vhalf():
                """V for 8 heads -> [128 s, sc, 8 h, 128]: cols 0:64 ones
                (row 0 of the PV result = softmax sums; rows 1:64 duplicate
                it, unread), head output in rows 64:128 -- 64-partition DVE
                ops must start at partition 0 or 64."""
                vt = vtp.tile([128, NSC, 8, 128], BF16, tag="vt")
                nc.vector.memset(vt[:, :, :, 0:64], 1.0)
                return vt

            def v_unit(vt, wvT, sc):
                p = psw.tile([128, 512], F32, tag="work")
                for dc in range(NDC):
                    nc.tensor.matmul(
                        p, xT[:, dc, sc * 128:(sc + 1) * 128], wvT[:, dc, :],
                        start=(dc == 0), stop=(dc == NDC - 1))
                nc.vector.tensor_copy(
                    out=vt[:, sc, :, 64:128],
                    in_=p.rearrange("p (h c) -> p h c", h=8))

            def attention(g, qt, kt, vt, fillers):
                fillers = list(fillers)
                fi = 0
                for qh in range(2):
                    qs = slice(qh * 512, (qh + 1) * 512)
                    pv0 = psv.tile([128, 512], F32, tag="pv")
                    pv1 = psv.tile([128, 512], F32, tag="pv")
                    for sc in range(NSC):
                        lg = psl.tile([128, 2, 512], F32, tag="log")
                        nc.tensor.matmul(
                            lg[:, 0, :], kt[0:64, sc * 128:(sc + 1) * 128],
                            qt[0:64, qs], start=True, stop=True)
                        nc.tensor.matmul(
                            lg[:, 1, :], kt[64:128, sc * 128:(sc + 1) * 128],
                            qt[64:128, qs], start=True, stop=True)
                        pt = ptp.tile([128, 2, 512], BF16, tag="pt")
                        nc.scalar.activation(
                            out=pt.rearrange("p a b -> p (a b)"),
                            in_=lg.rearrange("p a b -> p (a b)"),
                            func=EXP, scale=SCALE)
                        j = 2 * (g % 4)
                        nc.tensor.matmul(
                            pv0, vt[:, sc, j, :], pt[:, 0, :],
                            start=(sc == 0), stop=(sc == NSC - 1))
                        nc.tensor.matmul(
                            pv1, vt[:, sc, j + 1, :], pt[:, 1, :],
                            start=(sc == 0), stop=(sc == NSC - 1))
                        if sc % 2 == 1 and fi < len(fillers):
                            fillers[fi]()
                            fi += 1
                    for hloc, pv in ((0, pv0), (1, pv1)):
                        # row 0 = sums. PE/PSUM-free normalize: copy to SBUF
                        # (frees the PSUM bank), reciprocal row 0, gpsimd
                        # partition broadcast, DVE multiply, DMA shift.
                        pvs = sp.tile([128, 512], BF16, tag="pvs")
                        with nc.allow_low_precision(
                                reason="softmax weights tolerate bf16"):
                            nc.vector.tensor_copy(out=pvs, in_=pv)
                        rec1 = sp.tile([1, 512], BF16, tag="rec1")
                        with nc.allow_low_precision(
                                reason="softmax denom reciprocal fits bf16"):
                            nc.vector.reciprocal(out=rec1, in_=pvs[0:1, :])
                        rec = sp.tile([128, 512], BF16, tag="rec")
                        nc.gpsimd.partition_broadcast(rec[:, :], rec1[:, :])
                        tmp = sp.tile([128, 512], BF16, tag="tmp")
                        nc.vector.tensor_mul(
                            out=tmp[64:128, :], in0=pvs[64:128, :],
                            in1=rec[64:128, :])
                        nc.sync.dma_start(
                            out=catT[hloc * 64:(hloc + 1) * 64, g, qs],
                            in_=tmp[64:128, :])
                while fi < len(fillers):
                    fillers[fi]()
                    fi += 1

            # ---- prologue: V half 0, QT/KT group 0 ----
            vt = new_vhalf()
            for sc in range(NSC):
                v_unit(vt, wvT, sc)
            qt = qktp.tile([128, TQ], BF16, tag="qt")
            kt = qktp.tile([128, T], BF16, tag="kt")
            proj_unit(wq0, qt, 0)
            proj_unit(wq0, qt, 1)
            for w in range(4):
                proj_unit(wk0, kt, w)

            vt_next = None
            for g in range(NG):
                fillers = []
                if g < NG - 1:
                    wqn, wkn = load_wqk(g + 1)
                    qt_n = qktp.tile([128, TQ], BF16, tag="qt")
                    kt_n = qktp.tile([128, T], BF16, tag="kt")
                    for w in range(2):
                        fillers.append(
                            lambda w_=wqn, d=qt_n, i=w: proj_unit(w_, d, i))
                    for w in range(4):
                        fillers.append(
                            lambda w_=wkn, d=kt_n, i=w: proj_unit(w_, d, i))
                if g == 1:
                    wvT = wvp.tile([128, NDC, 512], BF16, tag="wvT")
                    nc.sync.dma_start(out=wvT, in_=wv[128:256, :])
                    vt_next = new_vhalf()
                if g in (2, 3):
                    lo = 0 if g == 2 else NSC // 2
                    for sc in range(lo, lo + NSC // 2):
                        fillers.append(lambda v=vt_next, w=wvT, s=sc: v_unit(v, w, s))
                attention(g, qt, kt, vt, fillers)
                if g == 3:
                    vt = vt_next
                if g < NG - 1:
                    qt, kt = qt_n, kt_n

            # ---- final projection ----
            for qb in range(8):
                yt = yp.tile([128, D], F32, tag="yt")
                p0 = psw.tile([128, 512], F32, tag="work")
                p1 = psw.tile([128, 512], F32, tag="work")
                for gg in range(NG):
                    nc.tensor.matmul(
                        p0, catT[:, gg, qb * 128:(qb + 1) * 128],
                        woT[:, gg, 0:512], start=(gg == 0), stop=(gg == NG - 1))
                    nc.tensor.matmul(
                        p1, catT[:, gg, qb * 128:(qb + 1) * 128],
                        woT[:, gg, 512:1024], start=(gg == 0), stop=(gg == NG - 1))
                nc.vector.tensor_add(out=yt[:, 0:512], in0=p0, in1=bias[:, 0:512])
                nc.vector.tensor_add(out=yt[:, 512:1024], in0=p1, in1=bias[:, 512:1024])
                nc.sync.dma_start(out=y[qb * 128:(qb + 1) * 128, :], in_=yt)

    nc.compile()
    return nc


_CACHE = {}


def _make_in_maps(ins):
    bf = ml_dtypes.bfloat16
    x = np.asarray(ins["x"], dtype=np.float32)
    Wq2 = np.asarray(ins["Wq"], dtype=np.float32).reshape(D, D)
    Wk2 = np.asarray(ins["Wk"], dtype=np.float32).reshape(D, D)
    Wv2 = np.asarray(ins["Wv"], dtype=np.float32).reshape(D, D)
    Wo2 = np.asarray(ins["Wo"], dtype=np.float32)
    # per-group packed W^T: wq_r[g*128+p, dc*128+j] = Wq2[g*128+j, dc*128+p]
    wq_r = np.ascontiguousarray(
        Wq2.reshape(8, 128, 8, 128).transpose(0, 3, 2, 1).reshape(D, D).astype(bf))
    wk_r = np.ascontiguousarray(
        Wk2.reshape(8, 128, 8, 128).transpose(0, 3, 2, 1).reshape(D, D).astype(bf))
    # wv_r[hh*128+p, dc*512+c] = Wv2[hh*512+c, dc*128+p]
    wv_r = np.ascontiguousarray(
        Wv2.reshape(2, 512, 8, 128).transpose(0, 3, 2, 1).reshape(256, 4096).astype(bf))
    wo_r = np.ascontiguousarray(Wo2.T.astype(bf))
    bo2 = np.ascontiguousarray(
        np.asarray(ins["bo"], dtype=np.float32).reshape(1, D))
    xT = x.transpose(0, 2, 1).astype(bf)  # [4, 1024, 2048]
    in_maps = []
    for c in range(8):
        b, h = c // 2, c % 2
        if h == 0:
            xtc = np.ascontiguousarray(xT[b])
        else:
            xtc = np.ascontiguousarray(
                np.concatenate([xT[b][:, TQ:], xT[b][:, :TQ]], axis=1))
        in_maps.append({"xt": xtc, "wq": wq_r, "wk": wk_r, "wv": wv_r,
                        "wo": wo_r, "bo": bo2})
    return in_maps


def kernel(x, Wq, Wk, Wv, Wo, bo):
    if "nc" not in _CACHE:
        _CACHE["nc"] = build_nc()
    nc = _CACHE["nc"]
    in_maps = _make_in_maps(
        {"x": x, "Wq": Wq, "Wk": Wk, "Wv": Wv, "Wo": Wo, "bo": bo})
    res = run_bass_kernel_spmd(nc, in_maps, core_ids=list(range(8)))
    out = np.empty((4, T, D), dtype=np.float32)
    for c in range(8):
        b, h = c // 2, c % 2
        out[b, h * TQ:(h + 1) * TQ] = res.results[c]["y"]
    return out
